# revision 2
# baseline (speedup 1.0000x reference)
"""Bahdanau attention Trainium2 kernel.

reference:
    proj_v = values @ W1 + b1            [B,T,U]
    proj_q = (query @ W2 + b2)[:,None,:] [B,1,U]
    score  = tanh(proj_v + proj_q) @ V + bV   [B,T,1]
    attn   = softmax(score, axis=1)
    ctx    = sum_t attn * values         [B,D]
    returns (ctx, attn)

B=32, T=2048, D=U=512. Data-parallel over batch: 8 cores x 4 batches.

Per-core dataflow (all within one NeuronCore, Tile-scheduled):
  - host precomputes pb[u,b] = (query@W2 + b2 + b1).T (tiny), so the kernel
    only consumes values, W1, V, pb. bV cancels in the softmax and is dropped.
  - natural-layout load of values[b] (t on partitions), PE-transpose to
    vT (d on partitions) with float32r rounding copies,
  - main matmul proj_vT[u,t] = W1.T @ values[b].T in float32r (full PE rate,
    ~1.4e-4 rel err), tanh+bias on ACT into bf16,
  - score = V.T @ tanh on PE (bf16), softmax on DVE/ACT/GPSIMD,
  - context via DVE per-partition scale of the natural tiles + ones-vector
    PE matmul for the partition (t) reduction.
"""

import os
import numpy as np

B, T, D, U = 32, 2048, 512, 512
N_CORES = 8
BPC = B // N_CORES  # batches per core
DC = D // 128       # d chunks
UC = U // 128       # u chunks
TT = T // 128       # t tiles of 128
HALF = T // 2       # DMA granule rows

_cache = {}


def _build():
    import concourse.mybir as mybir
    import concourse.tile as tile
    from concourse import bacc
    from concourse.masks import make_identity
    from contextlib import ExitStack

    f32 = mybir.dt.float32
    f32r = mybir.dt.float32r
    bf16 = mybir.dt.bfloat16
    TANH = mybir.ActivationFunctionType.Tanh
    EXP = mybir.ActivationFunctionType.Exp
    X = mybir.AxisListType.X

    nc = bacc.Bacc("TRN2", target_bir_lowering=False, debug=False,
                   num_devices=N_CORES)
    vals = nc.declare_dram_parameter("vals", [BPC, T, D], f32, isOutput=False)
    w1 = nc.declare_dram_parameter("w1", [D, U], f32r, isOutput=False)
    vbf = nc.declare_dram_parameter("vbf", [U], bf16, isOutput=False)
    pb = nc.declare_dram_parameter("pb", [U, BPC], f32, isOutput=False)
    attn_o = nc.declare_dram_parameter("attn_o", [BPC, T], f32, isOutput=True)
    ctx_o = nc.declare_dram_parameter("ctx_o", [BPC, D], f32, isOutput=True)

    with tile.TileContext(nc) as tc:
        with ExitStack() as ctx:
            singles = ctx.enter_context(tc.tile_pool(name="singles", bufs=1))
            natp = ctx.enter_context(tc.tile_pool(name="natp", bufs=3))
            vtp = ctx.enter_context(tc.tile_pool(name="vtp", bufs=2))
            thp = ctx.enter_context(tc.tile_pool(name="thp", bufs=2))
            scp = ctx.enter_context(tc.tile_pool(name="scp", bufs=4))
            rows = ctx.enter_context(tc.tile_pool(name="rows", bufs=2))
            ps_main = ctx.enter_context(tc.tile_pool(name="ps_main", bufs=2, space="PSUM"))
            ps_tr = ctx.enter_context(tc.tile_pool(name="ps_tr", bufs=2, space="PSUM"))
            ps_sc = ctx.enter_context(tc.tile_pool(name="ps_sc", bufs=2, space="PSUM"))

            # ---- constants ----
            w1sb = singles.tile([128, DC, U], f32r, name="w1sb")
            nc.sync.dma_start(out=w1sb[:], in_=w1.rearrange("(c p) u -> p c u", p=128))
            vsb = singles.tile([128, UC], bf16, name="vsb")
            nc.sync.dma_start(out=vsb[:], in_=vbf.rearrange("(c p) -> p c", p=128))
            pbsb = singles.tile([128, UC, BPC], f32, name="pbsb")
            nc.sync.dma_start(out=pbsb[:], in_=pb.rearrange("(c p) b -> p c b", p=128))
            ident = singles.tile([128, 128], f32, name="ident")
            make_identity(nc, ident)
            ones = singles.tile([128, 1], f32, name="ones")
            nc.vector.memset(ones[:], 1.0)
            onesr = singles.tile([128, 1], f32r, name="onesr")
            nc.vector.tensor_copy(onesr[:], ones[:])

            for b in range(BPC):
                # ---- load natural tiles (t on partitions), 2 halves ----
                nat = []
                for h in range(2):
                    nt = natp.tile([128, TT // 2, D], f32, tag="nat",
                                   name=f"nat_{b}_{h}")
                    nc.sync.dma_start(
                        out=nt[:],
                        in_=vals[b, h * HALF:(h + 1) * HALF, :]
                        .rearrange("(g p) d -> p g d", p=128))
                    nat.append(nt)

                # ---- PE transpose into vT (d on partitions), f32r ----
                vt = vtp.tile([128, DC, T], f32r, tag="vt", name=f"vt_{b}")
                ncopy = 0
                for dc in range(DC):
                    for gg in range(4):  # groups of 4 t-tiles = 512 t
                        pst = ps_tr.tile([128, 512], f32, tag="tr",
                                         name=f"pst_{b}_{dc}_{gg}")
                        for j in range(4):
                            tt = 4 * gg + j
                            h, g = divmod(tt, TT // 2)
                            nc.tensor.transpose(
                                pst[:, j * 128:(j + 1) * 128],
                                nat[h][:, g, dc * 128:(dc + 1) * 128],
                                ident[:])
                        dst = vt[:, dc, gg * 512:(gg + 1) * 512]
                        if ncopy % 2 == 0:
                            nc.vector.tensor_copy(dst, pst[:])
                        else:
                            nc.scalar.copy(dst, pst[:])
                        ncopy += 1

                # ---- main matmul + tanh (bf16 out) ----
                th = thp.tile([128, UC, T], bf16, tag="th", name=f"th_{b}")
                for ut in range(UC):
                    for h2 in range(2):
                        psm = ps_main.tile([128, 1024], f32, tag="main",
                                           name=f"psm_{b}_{ut}_{h2}")
                        for dc in range(DC):
                            for c2 in range(2):
                                tcoff = h2 * 1024 + c2 * 512
                                nc.tensor.matmul(
                                    psm[:, c2 * 512:(c2 + 1) * 512],
                                    w1sb[:, dc, ut * 128:(ut + 1) * 128],
                                    vt[:, dc, tcoff:tcoff + 512],
                                    start=(dc == 0), stop=(dc == DC - 1))
                        nc.scalar.activation(
                            out=th[:, ut, h2 * 1024:(h2 + 1) * 1024],
                            in_=psm[:], func=TANH,
                            bias=pbsb[:, ut, b:b + 1], scale=1.0)

                # ---- score = V.T @ tanh  -> [1, T] ----
                score = rows.tile([1, T], f32, tag="score", name=f"score_{b}")
                for tch in range(4):
                    pss = ps_sc.tile([1, 512], f32, tag="sc",
                                     name=f"pss_{b}_{tch}")
                    for uc in range(UC):
                        nc.tensor.matmul(
                            pss[:], vsb[:, uc:uc + 1],
                            th[:, uc, tch * 512:(tch + 1) * 512],
                            start=(uc == 0), stop=(uc == UC - 1))
                    nc.scalar.copy(score[:, tch * 512:(tch + 1) * 512], pss[:])

                # ---- softmax on [1, T], in place on score ----
                mx = rows.tile([1, 1], f32, tag="mx", name=f"mx_{b}")
                nc.vector.reduce_max(mx[:], score[:], axis=X)
                nmx = rows.tile([1, 1], f32, tag="nmx", name=f"nmx_{b}")
                nc.vector.tensor_scalar_mul(nmx[:], mx[:], -1.0)
                nc.scalar.activation(out=score[:], in_=score[:], func=EXP,
                                     bias=nmx[:], scale=1.0)
                sm = rows.tile([1, 1], f32, tag="sm", name=f"sm_{b}")
                nc.vector.reduce_sum(sm[:], score[:], axis=X)
                rsum = rows.tile([1, 1], f32, tag="rsum", name=f"rsum_{b}")
                nc.vector.reciprocal(rsum[:], sm[:])
                nc.gpsimd.tensor_scalar_mul(score[:], score[:], rsum[:])
                nc.sync.dma_start(out=attn_o[b:b + 1, :], in_=score[:])

                # ---- attn row -> columns [128, TT] via DRAM bounce + PE T ----
                attn16 = rows.tile([TT, 128], f32, tag="attn16",
                                   name=f"attn16_{b}")
                nc.sync.dma_start(
                    out=attn16[:],
                    in_=attn_o[b:b + 1, :].rearrange("o (tt p) -> (o tt) p", p=128))
                psa = ps_tr.tile([128, TT], f32, tag="tr", name=f"psa_{b}")
                nc.tensor.transpose(psa[:], attn16[:], ident[0:TT, 0:TT])
                acol = rows.tile([128, TT], f32, tag="acol", name=f"acol_{b}")
                nc.vector.tensor_copy(acol[:], psa[:])

                # ---- context: scale nat tiles, ones-matmul reduce over t ----
                psc = ps_sc.tile([1, D], f32, tag="sc", name=f"psc_{b}")
                for tt in range(TT):
                    h, g = divmod(tt, TT // 2)
                    sc = scp.tile([128, D], f32r, tag="scaled",
                                  name=f"sc_{b}_{tt}")
                    nc.vector.tensor_scalar_mul(sc[:], nat[h][:, g, :],
                                                acol[:, tt:tt + 1])
                    nc.tensor.matmul(psc[:], onesr[:], sc[:],
                                     start=(tt == 0), stop=(tt == TT - 1))
                ctxr = rows.tile([1, D], f32, tag="ctxr", name=f"ctxr_{b}")
                nc.scalar.copy(ctxr[:], psc[:])
                nc.sync.dma_start(out=ctx_o[b:b + 1, :], in_=ctxr[:])

    nc.compile()
    return nc


def _get_nc():
    if "nc" not in _cache:
        _cache["nc"] = _build()
    return _cache["nc"]


def kernel(query, values, W1, b1, W2, b2, V, bV):
    import ml_dtypes
    from concourse.bass_utils import run_bass_kernel_spmd

    query = np.asarray(query, dtype=np.float32)
    values = np.asarray(values, dtype=np.float32)
    W1 = np.asarray(W1, dtype=np.float32)
    b1 = np.asarray(b1, dtype=np.float32)
    W2 = np.asarray(W2, dtype=np.float32)
    b2 = np.asarray(b2, dtype=np.float32)
    V = np.asarray(V, dtype=np.float32)

    # host-side tiny prep: pb[u, b] = (query @ W2 + b2 + b1)[b, u]
    pq = query @ W2 + b2[None, :] + b1[None, :]   # [B, U]
    pb_full = np.ascontiguousarray(pq.T)          # [U, B]
    vbf = V[:, 0].astype(ml_dtypes.bfloat16)      # [U]

    in_maps = []
    for c in range(N_CORES):
        sl = slice(c * BPC, (c + 1) * BPC)
        in_maps.append({
            "vals": np.ascontiguousarray(values[sl]),
            "w1": W1,
            "vbf": vbf,
            "pb": np.ascontiguousarray(pb_full[:, sl]),
        })

    nc = _get_nc()
    trace = os.environ.get("BASS_KERNEL_TRACE") == "1"
    if trace:
        try:
            import tracehelper
            tracehelper.install()
        except Exception:
            trace = False
    res = run_bass_kernel_spmd(nc, in_maps, list(range(N_CORES)), trace=trace)
    _cache["last_exec_time_ns"] = res.exec_time_ns

    context = np.empty((B, D), dtype=np.float32)
    attn = np.empty((B, T, 1), dtype=np.float32)
    for c in range(N_CORES):
        sl = slice(c * BPC, (c + 1) * BPC)
        context[sl] = res.results[c]["ctx_o"]
        attn[sl] = res.results[c]["attn_o"][:, :, None]
    return (context, attn)


# revision 3
# speedup vs baseline: 1.1013x; 1.1013x over previous
"""Bahdanau attention Trainium2 kernel.

reference:
    proj_v = values @ W1 + b1            [B,T,U]
    proj_q = (query @ W2 + b2)[:,None,:] [B,1,U]
    score  = tanh(proj_v + proj_q) @ V + bV   [B,T,1]
    attn   = softmax(score, axis=1)
    ctx    = sum_t attn * values         [B,D]
    returns (ctx, attn)

B=32, T=2048, D=U=512. Data-parallel over batch: 8 cores x 4 batches.

Per-core dataflow (one NeuronCore, Tile-scheduled):
  - host precomputes pb[u,b] = (query@W2 + b2 + b1).T (tiny); bV cancels in
    softmax and is dropped.
  - values DMA'd as float32r (rounded, ~1e-4) in natural layout (t on
    partitions), PE-transposed (f32r transpose mode, exact on rounded data)
    to vT (d on partitions),
  - main matmul proj_vT[u,t] = W1.T @ values[b].T in float32r at full PE
    rate, tanh+bias on ACT into bf16,
  - score = V.T @ tanh on PE (bf16), softmax on DVE/ACT/GPSIMD,
  - context: DVE per-partition scale of the natural tiles by attn columns
    (attn transposed via a tiny PE transpose through a DRAM bounce), then a
    ones-vector PE matmul reduces over t.
  - emission is software-pipelined: context of batch b-1 is emitted after
    the main matmuls of batch b so the in-order PE queue never waits on
    batch b-1's softmax tail.
"""

import os
import numpy as np

B, T, D, U = 32, 2048, 512, 512
N_CORES = 8
BPC = B // N_CORES  # batches per core
DC = D // 128       # d chunks
UC = U // 128       # u chunks
TT = T // 128       # t tiles of 128
HALF = T // 2

_cache = {}


def _build():
    import concourse.mybir as mybir
    import concourse.tile as tile
    from concourse import bacc
    from concourse.masks import make_identity
    from contextlib import ExitStack

    f32 = mybir.dt.float32
    f32r = mybir.dt.float32r
    bf16 = mybir.dt.bfloat16
    TANH = mybir.ActivationFunctionType.Tanh
    EXP = mybir.ActivationFunctionType.Exp
    X = mybir.AxisListType.X

    nc = bacc.Bacc("TRN2", target_bir_lowering=False, debug=False,
                   num_devices=N_CORES)
    vals = nc.declare_dram_parameter("vals", [BPC, T, D], f32r, isOutput=False)
    w1 = nc.declare_dram_parameter("w1", [D, U], f32r, isOutput=False)
    vbf = nc.declare_dram_parameter("vbf", [U], bf16, isOutput=False)
    pb = nc.declare_dram_parameter("pb", [U, BPC], f32, isOutput=False)
    attn_o = nc.declare_dram_parameter("attn_o", [BPC, T], f32, isOutput=True)
    ctx_o = nc.declare_dram_parameter("ctx_o", [BPC, D], f32, isOutput=True)

    with tile.TileContext(nc) as tc:
        with ExitStack() as ctx:
            singles = ctx.enter_context(tc.tile_pool(name="singles", bufs=1))
            natp = ctx.enter_context(tc.tile_pool(name="natp", bufs=4))
            vtp = ctx.enter_context(tc.tile_pool(name="vtp", bufs=2))
            thp = ctx.enter_context(tc.tile_pool(name="thp", bufs=2))
            scp = ctx.enter_context(tc.tile_pool(name="scp", bufs=4))
            rows = ctx.enter_context(tc.tile_pool(name="rows", bufs=2))
            small = ctx.enter_context(tc.tile_pool(name="small", bufs=2))
            ps_main = ctx.enter_context(tc.tile_pool(name="ps_main", bufs=2, space="PSUM"))
            ps_tr = ctx.enter_context(tc.tile_pool(name="ps_tr", bufs=2, space="PSUM"))
            ps_sc = ctx.enter_context(tc.tile_pool(name="ps_sc", bufs=2, space="PSUM"))

            # ---- constants ----
            w1sb = singles.tile([128, DC, U], f32r, name="w1sb")
            nc.scalar.dma_start(out=w1sb[:], in_=w1.rearrange("(c p) u -> p c u", p=128))
            vsb = singles.tile([128, UC], bf16, name="vsb")
            nc.scalar.dma_start(out=vsb[:], in_=vbf.rearrange("(c p) -> p c", p=128))
            pbsb = singles.tile([128, UC, BPC], f32, name="pbsb")
            nc.scalar.dma_start(out=pbsb[:], in_=pb.rearrange("(c p) b -> p c b", p=128))
            identf = singles.tile([128, 128], f32, name="identf")
            make_identity(nc, identf)
            identr = singles.tile([128, 128], f32r, name="identr")
            nc.vector.tensor_copy(identr[:], identf[:])
            ones = singles.tile([128, 1], f32, name="ones")
            nc.vector.memset(ones[:], 1.0)
            onesr = singles.tile([128, 1], f32r, name="onesr")
            nc.vector.tensor_copy(onesr[:], ones[:])

            nats = {}
            acols = {}

            def stage_load(b):
                nat = []
                for h in range(2):
                    nt = natp.tile([128, TT // 2, D], f32r, tag="nat",
                                   name=f"nat_{b}_{h}")
                    nc.sync.dma_start(
                        out=nt[:],
                        in_=vals[b, h * HALF:(h + 1) * HALF, :]
                        .rearrange("(g p) d -> p g d", p=128))
                    nat.append(nt)
                nats[b] = nat

            def stage_transpose(b):
                nat = nats[b]
                vt = vtp.tile([128, DC, T], f32r, tag="vt", name=f"vt_{b}")
                ncopy = 0
                for dc in range(DC):
                    for gg in range(4):
                        pst = ps_tr.tile([128, 512], f32r, tag="tr",
                                         name=f"pst_{b}_{dc}_{gg}")
                        for j in range(4):
                            tt = 4 * gg + j
                            h, g = divmod(tt, TT // 2)
                            nc.tensor.transpose(
                                pst[:, j * 128:(j + 1) * 128],
                                nat[h][:, g, dc * 128:(dc + 1) * 128],
                                identr[:])
                        dst = vt[:, dc, gg * 512:(gg + 1) * 512]
                        if ncopy % 2 == 0:
                            nc.vector.tensor_copy(dst, pst[:])
                        else:
                            nc.scalar.copy(dst, pst[:])
                        ncopy += 1
                return vt

            def stage_main(b, vt):
                th = thp.tile([128, UC, T], bf16, tag="th", name=f"th_{b}")
                for ut in range(UC):
                    for h2 in range(2):
                        psm = ps_main.tile([128, 1024], f32, tag="main",
                                           name=f"psm_{b}_{ut}_{h2}")
                        for dc in range(DC):
                            for c2 in range(2):
                                tcoff = h2 * 1024 + c2 * 512
                                nc.tensor.matmul(
                                    psm[:, c2 * 512:(c2 + 1) * 512],
                                    w1sb[:, dc, ut * 128:(ut + 1) * 128],
                                    vt[:, dc, tcoff:tcoff + 512],
                                    start=(dc == 0), stop=(dc == DC - 1))
                        nc.scalar.activation(
                            out=th[:, ut, h2 * 1024:(h2 + 1) * 1024],
                            in_=psm[:], func=TANH,
                            bias=pbsb[:, ut, b:b + 1], scale=1.0)
                return th

            def stage_score_softmax(b, th):
                score = rows.tile([1, T], f32, tag="score", name=f"score_{b}")
                for tch in range(4):
                    pss = ps_sc.tile([1, 512], f32, tag="sc",
                                     name=f"pss_{b}_{tch}")
                    for uc in range(UC):
                        nc.tensor.matmul(
                            pss[:], vsb[:, uc:uc + 1],
                            th[:, uc, tch * 512:(tch + 1) * 512],
                            start=(uc == 0), stop=(uc == UC - 1))
                    nc.scalar.copy(score[:, tch * 512:(tch + 1) * 512], pss[:])
                mx = small.tile([1, 1], f32, tag="mx", name=f"mx_{b}")
                nc.vector.reduce_max(mx[:], score[:], axis=X)
                nmx = small.tile([1, 1], f32, tag="nmx", name=f"nmx_{b}")
                nc.vector.tensor_scalar_mul(nmx[:], mx[:], -1.0)
                nc.scalar.activation(out=score[:], in_=score[:], func=EXP,
                                     bias=nmx[:], scale=1.0)
                sm = small.tile([1, 1], f32, tag="sm", name=f"sm_{b}")
                nc.vector.reduce_sum(sm[:], score[:], axis=X)
                rsum = small.tile([1, 1], f32, tag="rsum", name=f"rsum_{b}")
                nc.vector.reciprocal(rsum[:], sm[:])
                nc.gpsimd.tensor_scalar_mul(score[:], score[:], rsum[:])
                nc.scalar.dma_start(out=attn_o[b:b + 1, :], in_=score[:])

            def stage_acol(b):
                # attn row -> columns [128, TT] via DRAM bounce + PE transpose
                attn16 = small.tile([TT, 128], f32, tag="attn16",
                                    name=f"attn16_{b}")
                nc.scalar.dma_start(
                    out=attn16[:],
                    in_=attn_o[b:b + 1, :].rearrange("o (tt p) -> (o tt) p", p=128))
                psa = ps_tr.tile([128, TT], f32, tag="tr", name=f"psa_{b}")
                nc.tensor.transpose(psa[:], attn16[:], identf[0:TT, 0:TT])
                acol = small.tile([128, TT], f32, tag="acol", name=f"acol_{b}")
                nc.vector.tensor_copy(acol[:], psa[:])
                acols[b] = acol

            def stage_ctx(b):
                nat, acol = nats[b], acols[b]
                psc = ps_sc.tile([1, D], f32, tag="sc", name=f"psc_{b}")
                for tt in range(TT):
                    h, g = divmod(tt, TT // 2)
                    sc = scp.tile([128, D], f32r, tag="scaled",
                                  name=f"sc_{b}_{tt}")
                    nc.vector.tensor_scalar_mul(
                        sc[:], nat[h][:, g, :].bitcast(f32),
                        acol[:, tt:tt + 1])
                    nc.tensor.matmul(psc[:], onesr[:], sc[:],
                                     start=(tt == 0), stop=(tt == TT - 1))
                ctxr = small.tile([1, D], f32, tag="ctxr", name=f"ctxr_{b}")
                nc.scalar.copy(ctxr[:], psc[:])
                nc.scalar.dma_start(out=ctx_o[b:b + 1, :], in_=ctxr[:])

            # ---- software-pipelined emission ----
            for b in range(BPC):
                stage_load(b)
                vt = stage_transpose(b)
                th = stage_main(b, vt)
                if b >= 1:
                    stage_ctx(b - 1)
                stage_score_softmax(b, th)
                stage_acol(b)
            stage_ctx(BPC - 1)

    nc.compile()
    return nc


def _get_nc():
    if "nc" not in _cache:
        _cache["nc"] = _build()
    return _cache["nc"]


def kernel(query, values, W1, b1, W2, b2, V, bV):
    import ml_dtypes
    from concourse.bass_utils import run_bass_kernel_spmd

    query = np.asarray(query, dtype=np.float32)
    values = np.asarray(values, dtype=np.float32)
    W1 = np.asarray(W1, dtype=np.float32)
    b1 = np.asarray(b1, dtype=np.float32)
    W2 = np.asarray(W2, dtype=np.float32)
    b2 = np.asarray(b2, dtype=np.float32)
    V = np.asarray(V, dtype=np.float32)

    pq = query @ W2 + b2[None, :] + b1[None, :]   # [B, U]
    pb_full = np.ascontiguousarray(pq.T)          # [U, B]
    vbf = V[:, 0].astype(ml_dtypes.bfloat16)      # [U]

    in_maps = []
    for c in range(N_CORES):
        sl = slice(c * BPC, (c + 1) * BPC)
        in_maps.append({
            "vals": np.ascontiguousarray(values[sl]),
            "w1": W1,
            "vbf": vbf,
            "pb": np.ascontiguousarray(pb_full[:, sl]),
        })

    nc = _get_nc()
    trace = os.environ.get("BASS_KERNEL_TRACE") == "1"
    if trace:
        try:
            import tracehelper
            tracehelper.install()
        except Exception:
            trace = False
    res = run_bass_kernel_spmd(nc, in_maps, list(range(N_CORES)), trace=trace)
    _cache["last_exec_time_ns"] = res.exec_time_ns

    context = np.empty((B, D), dtype=np.float32)
    attn = np.empty((B, T, 1), dtype=np.float32)
    for c in range(N_CORES):
        sl = slice(c * BPC, (c + 1) * BPC)
        context[sl] = res.results[c]["ctx_o"]
        attn[sl] = res.results[c]["attn_o"][:, :, None]
    return (context, attn)


# revision 5
# speedup vs baseline: 1.8885x; 1.7147x over previous
"""Bahdanau attention Trainium2 kernel.

reference:
    proj_v = values @ W1 + b1            [B,T,U]
    proj_q = (query @ W2 + b2)[:,None,:] [B,1,U]
    score  = tanh(proj_v + proj_q) @ V + bV   [B,T,1]
    attn   = softmax(score, axis=1)
    ctx    = sum_t attn * values         [B,D]
    returns (ctx, attn)

B=32, T=2048, D=U=512. Data-parallel over batch: 8 cores x 4 batches.

Per-core dataflow (one NeuronCore, Tile-scheduled):
  - host precomputes pb[u,b] = (query@W2 + b2 + b1).T (tiny); bV cancels in
    softmax and is dropped.
  - values DMA'd as float32r (rounded, ~1e-4) in natural layout (t on
    partitions, granules of 512 t), PE-transposed (f32r transpose mode,
    exact on rounded data) to vT (d on partitions),
  - main matmul proj_vT[u,t] = W1.T @ values[b].T in float32r at full PE
    rate, tanh+bias on ACT into bf16,
  - score = V.T @ tanh on PE -> [1,T] rows; softmax without max-shift
    (|score| <= sum|V| ~ 18, exp is safe in fp32): exp on ACT; the 1/sum
    normalization folds into the final context copy and a row-scale,
  - attn columns [128,16] come from 16 tiny [1,128] PE transposes of the
    raw score row, exp'd on ACT straight out of PSUM; context = DVE
    per-partition scale of natural granules + ones-vector PE matmul
    reduction over t, scaled by 1/sum in the PSUM->SBUF copy.
"""

import os
import numpy as np

B, T, D, U = 32, 2048, 512, 512
N_CORES = 8
BPC = B // N_CORES  # batches per core
DC = D // 128       # d chunks
UC = U // 128       # u chunks
TT = T // 128       # t tiles of 128
NG = 4              # nat granules per batch
GT = TT // NG       # t tiles per granule

_cache = {}


def _build():
    import concourse.mybir as mybir
    import concourse.tile as tile
    from concourse import bacc
    from concourse.masks import make_identity
    from contextlib import ExitStack

    f32 = mybir.dt.float32
    f32r = mybir.dt.float32r
    bf16 = mybir.dt.bfloat16
    TANH = mybir.ActivationFunctionType.Tanh
    EXP = mybir.ActivationFunctionType.Exp
    COPY = mybir.ActivationFunctionType.Copy
    X = mybir.AxisListType.X

    nc = bacc.Bacc("TRN2", target_bir_lowering=False, debug=False,
                   num_devices=N_CORES)
    vals = nc.declare_dram_parameter("vals", [BPC, T, D], f32r, isOutput=False)
    w1 = nc.declare_dram_parameter("w1", [D, U], f32r, isOutput=False)
    vbf = nc.declare_dram_parameter("vbf", [U], bf16, isOutput=False)
    pb = nc.declare_dram_parameter("pb", [U, BPC], f32, isOutput=False)
    attn_o = nc.declare_dram_parameter("attn_o", [BPC, T], f32, isOutput=True)
    ctx_o = nc.declare_dram_parameter("ctx_o", [BPC, D], f32, isOutput=True)

    with tile.TileContext(nc) as tc:
        with ExitStack() as ctx:
            singles = ctx.enter_context(tc.tile_pool(name="singles", bufs=1))
            natp = ctx.enter_context(tc.tile_pool(name="natp", bufs=7))
            vtp = ctx.enter_context(tc.tile_pool(name="vtp", bufs=2))
            thp = ctx.enter_context(tc.tile_pool(name="thp", bufs=2))
            scp = ctx.enter_context(tc.tile_pool(name="scp", bufs=4))
            small = ctx.enter_context(tc.tile_pool(name="small", bufs=2))
            ps_main = ctx.enter_context(tc.tile_pool(name="ps_main", bufs=2, space="PSUM"))
            ps_tr = ctx.enter_context(tc.tile_pool(name="ps_tr", bufs=2, space="PSUM"))
            ps_sc = ctx.enter_context(tc.tile_pool(name="ps_sc", bufs=2, space="PSUM"))

            # ---- constants ----
            w1sb = singles.tile([128, DC, U], f32r, name="w1sb")
            nc.scalar.dma_start(out=w1sb[:], in_=w1.rearrange("(c p) u -> p c u", p=128))
            vsb = singles.tile([128, UC], bf16, name="vsb")
            nc.scalar.dma_start(out=vsb[:], in_=vbf.rearrange("(c p) -> p c", p=128))
            pbsb = singles.tile([128, UC, BPC], f32, name="pbsb")
            nc.scalar.dma_start(out=pbsb[:], in_=pb.rearrange("(c p) b -> p c b", p=128))
            identf = singles.tile([128, 128], f32, name="identf")
            make_identity(nc, identf)
            identr = singles.tile([128, 128], f32r, name="identr")
            nc.vector.tensor_copy(identr[:], identf[:])
            ones = singles.tile([128, 1], f32, name="ones")
            nc.vector.memset(ones[:], 1.0)
            onesr = singles.tile([128, 1], f32r, name="onesr")
            nc.vector.tensor_copy(onesr[:], ones[:])

            for b in range(BPC):
                # ---- load natural granules (t on partitions) ----
                nat = []
                for q in range(NG):
                    nt = natp.tile([128, GT, D], f32r, tag="nat",
                                   name=f"nat_{b}_{q}")
                    nc.sync.dma_start(
                        out=nt[:],
                        in_=vals[b, q * GT * 128:(q + 1) * GT * 128, :]
                        .rearrange("(g p) d -> p g d", p=128))
                    nat.append(nt)

                # ---- PE transpose into vT (d on partitions) ----
                vt = vtp.tile([128, DC, T], f32r, tag="vt", name=f"vt_{b}")
                ncopy = 0
                for dc in range(DC):
                    for q in range(NG):
                        pst = ps_tr.tile([128, 512], f32r, tag="tr",
                                         name=f"pst_{b}_{dc}_{q}")
                        for j in range(GT):
                            nc.tensor.transpose(
                                pst[:, j * 128:(j + 1) * 128],
                                nat[q][:, j, dc * 128:(dc + 1) * 128],
                                identr[:])
                        dst = vt[:, dc, q * 512:(q + 1) * 512]
                        if ncopy % 2 == 0:
                            nc.vector.tensor_copy(dst, pst[:])
                        else:
                            nc.scalar.copy(dst, pst[:])
                        ncopy += 1

                # ---- main matmul + tanh (bf16 out) ----
                th = thp.tile([128, UC, T], bf16, tag="th", name=f"th_{b}")
                for ut in range(UC):
                    for h2 in range(2):
                        psm = ps_main.tile([128, 1024], f32, tag="main",
                                           name=f"psm_{b}_{ut}_{h2}")
                        for dc in range(DC):
                            for c2 in range(2):
                                tcoff = h2 * 1024 + c2 * 512
                                nc.tensor.matmul(
                                    psm[:, c2 * 512:(c2 + 1) * 512],
                                    w1sb[:, dc, ut * 128:(ut + 1) * 128],
                                    vt[:, dc, tcoff:tcoff + 512],
                                    start=(dc == 0), stop=(dc == DC - 1))
                        nc.scalar.activation(
                            out=th[:, ut, h2 * 1024:(h2 + 1) * 1024],
                            in_=psm[:], func=TANH,
                            bias=pbsb[:, ut, b:b + 1], scale=1.0)

                # ---- score on PE -> scr row [1, T] (raw scores) ----
                scr = small.tile([1, T], f32, tag="scr", name=f"scr_{b}")
                for tch in range(4):
                    pss = ps_sc.tile([1, 512], f32, tag="sc",
                                     name=f"pss_{b}_{tch}")
                    for uc in range(UC):
                        nc.tensor.matmul(
                            pss[:], vsb[:, uc:uc + 1],
                            th[:, uc, tch * 512:(tch + 1) * 512],
                            start=(uc == 0), stop=(uc == UC - 1))
                    nc.scalar.copy(scr[:, tch * 512:(tch + 1) * 512], pss[:])

                # ---- attn columns: 16 tiny [1,128] PE transposes + exp ----
                psa = ps_sc.tile([128, TT], f32, tag="sc", name=f"psa_{b}")
                for tt in range(TT):
                    nc.tensor.transpose(psa[:, tt:tt + 1],
                                        scr[0:1, tt * 128:(tt + 1) * 128],
                                        identf[0:1, 0:1])
                acol = small.tile([128, TT], f32, tag="acol", name=f"acol_{b}")
                nc.scalar.activation(out=acol[:], in_=psa[:], func=EXP,
                                     bias=0.0, scale=1.0)

                # ---- row path: exp, sum, scale, attn out ----
                nc.scalar.activation(out=scr[:], in_=scr[:], func=EXP,
                                     bias=0.0, scale=1.0)
                sm = small.tile([1, 1], f32, tag="sm", name=f"sm_{b}")
                nc.vector.reduce_sum(sm[:], scr[:], axis=X)
                rsum = small.tile([1, 1], f32, tag="rsum", name=f"rsum_{b}")
                nc.vector.reciprocal(rsum[:], sm[:])
                nc.vector.tensor_scalar_mul(scr[:], scr[:], rsum[:])
                nc.scalar.dma_start(out=attn_o[b:b + 1, :], in_=scr[:])

                # ---- context (unnormalized, scaled by rsum in the copy) ----
                psc = ps_sc.tile([1, D], f32, tag="sc", name=f"psc_{b}")
                for tt in range(TT):
                    q, g = divmod(tt, GT)
                    sc = scp.tile([128, D], f32r, tag="scaled",
                                  name=f"sc_{b}_{tt}")
                    nc.vector.tensor_scalar_mul(
                        sc[:], nat[q][:, g, :].bitcast(f32),
                        acol[:, tt:tt + 1])
                    nc.tensor.matmul(psc[:], onesr[:], sc[:],
                                     start=(tt == 0), stop=(tt == TT - 1))
                ctxr = small.tile([1, D], f32, tag="ctxr", name=f"ctxr_{b}")
                nc.scalar.activation(out=ctxr[:], in_=psc[:], func=COPY,
                                     bias=0.0, scale=rsum[:])
                nc.scalar.dma_start(out=ctx_o[b:b + 1, :], in_=ctxr[:])

    nc.compile()
    return nc


def _get_nc():
    if "nc" not in _cache:
        _cache["nc"] = _build()
    return _cache["nc"]


def kernel(query, values, W1, b1, W2, b2, V, bV):
    import ml_dtypes
    from concourse.bass_utils import run_bass_kernel_spmd

    query = np.asarray(query, dtype=np.float32)
    values = np.asarray(values, dtype=np.float32)
    W1 = np.asarray(W1, dtype=np.float32)
    b1 = np.asarray(b1, dtype=np.float32)
    W2 = np.asarray(W2, dtype=np.float32)
    b2 = np.asarray(b2, dtype=np.float32)
    V = np.asarray(V, dtype=np.float32)

    pq = query @ W2 + b2[None, :] + b1[None, :]   # [B, U]
    pb_full = np.ascontiguousarray(pq.T)          # [U, B]
    vbf = V[:, 0].astype(ml_dtypes.bfloat16)      # [U]

    in_maps = []
    for c in range(N_CORES):
        sl = slice(c * BPC, (c + 1) * BPC)
        in_maps.append({
            "vals": np.ascontiguousarray(values[sl]),
            "w1": W1,
            "vbf": vbf,
            "pb": np.ascontiguousarray(pb_full[:, sl]),
        })

    nc = _get_nc()
    trace = os.environ.get("BASS_KERNEL_TRACE") == "1"
    if trace:
        try:
            import tracehelper
            tracehelper.install()
        except Exception:
            trace = False
    res = run_bass_kernel_spmd(nc, in_maps, list(range(N_CORES)), trace=trace)
    _cache["last_exec_time_ns"] = res.exec_time_ns

    context = np.empty((B, D), dtype=np.float32)
    attn = np.empty((B, T, 1), dtype=np.float32)
    for c in range(N_CORES):
        sl = slice(c * BPC, (c + 1) * BPC)
        context[sl] = res.results[c]["ctx_o"]
        attn[sl] = res.results[c]["attn_o"][:, :, None]
    return (context, attn)


# revision 6
# speedup vs baseline: 2.3078x; 1.2221x over previous
"""Bahdanau attention Trainium2 kernel.

reference:
    proj_v = values @ W1 + b1            [B,T,U]
    proj_q = (query @ W2 + b2)[:,None,:] [B,1,U]
    score  = tanh(proj_v + proj_q) @ V + bV   [B,T,1]
    attn   = softmax(score, axis=1)
    ctx    = sum_t attn * values         [B,D]
    returns (ctx, attn)

B=32, T=2048, D=U=512. Data-parallel over batch: 8 cores x 4 batches.

Per-core dataflow (one NeuronCore, Tile-scheduled):
  - host precomputes pb[u,b] = (query@W2 + b2 + b1).T (tiny); bV cancels in
    softmax and is dropped.
  - values DMA'd as float32r (rounded, ~1e-4) in natural layout (t on
    partitions, granules of 512 t), PE-transposed (f32r transpose mode,
    exact on rounded data) to vT (d on partitions),
  - main matmul proj_vT[u,t] = W1.T @ values[b].T in float32r at full PE
    rate, tanh+bias on ACT into bf16,
  - score = V.T @ tanh on PE -> [1,T] rows; softmax without max-shift
    (|score| <= sum|V| ~ 18, exp is safe in fp32): exp on ACT; the 1/sum
    normalization folds into the final context copy and a row-scale,
  - attn columns [128,16] come from 16 tiny [1,128] PE transposes of the
    raw score row, exp'd on ACT straight out of PSUM; context = DVE
    per-partition scale of natural granules + ones-vector PE matmul
    reduction over t, scaled by 1/sum in the PSUM->SBUF copy.
"""

import os
import numpy as np

B, T, D, U = 32, 2048, 512, 512
N_CORES = 8
BPC = B // N_CORES  # batches per core
DC = D // 128       # d chunks
UC = U // 128       # u chunks
TT = T // 128       # t tiles of 128
NG = 4              # nat granules per batch
GT = TT // NG       # t tiles per granule

_cache = {}


def _build():
    import concourse.mybir as mybir
    import concourse.tile as tile
    from concourse import bacc
    from concourse.masks import make_identity
    from contextlib import ExitStack

    f32 = mybir.dt.float32
    f32r = mybir.dt.float32r
    bf16 = mybir.dt.bfloat16
    TANH = mybir.ActivationFunctionType.Tanh
    EXP = mybir.ActivationFunctionType.Exp
    COPY = mybir.ActivationFunctionType.Copy
    X = mybir.AxisListType.X

    nc = bacc.Bacc("TRN2", target_bir_lowering=False, debug=False,
                   num_devices=N_CORES)
    vals = nc.declare_dram_parameter("vals", [BPC, T, D], f32r, isOutput=False)
    w1 = nc.declare_dram_parameter("w1", [D, U], f32r, isOutput=False)
    vbf = nc.declare_dram_parameter("vbf", [U], bf16, isOutput=False)
    pb = nc.declare_dram_parameter("pb", [U, BPC], f32, isOutput=False)
    attn_o = nc.declare_dram_parameter("attn_o", [BPC, T], f32, isOutput=True)
    ctx_o = nc.declare_dram_parameter("ctx_o", [BPC, D], f32, isOutput=True)

    with tile.TileContext(nc) as tc:
        with ExitStack() as ctx:
            singles = ctx.enter_context(tc.tile_pool(name="singles", bufs=1))
            natp = ctx.enter_context(tc.tile_pool(name="natp", bufs=7))
            vtp = ctx.enter_context(tc.tile_pool(name="vtp", bufs=2))
            thp = ctx.enter_context(tc.tile_pool(name="thp", bufs=2))
            small = ctx.enter_context(tc.tile_pool(name="small", bufs=2))
            ps_main = ctx.enter_context(tc.tile_pool(name="ps_main", bufs=2, space="PSUM"))
            ps_tr = ctx.enter_context(tc.tile_pool(name="ps_tr", bufs=2, space="PSUM"))
            ps_sc = ctx.enter_context(tc.tile_pool(name="ps_sc", bufs=2, space="PSUM"))

            # ---- constants ----
            w1sb = singles.tile([128, DC, U], f32r, name="w1sb")
            nc.scalar.dma_start(out=w1sb[:], in_=w1.rearrange("(c p) u -> p c u", p=128))
            vsb = singles.tile([128, UC], bf16, name="vsb")
            nc.scalar.dma_start(out=vsb[:], in_=vbf.rearrange("(c p) -> p c", p=128))
            pbsb = singles.tile([128, UC, BPC], f32, name="pbsb")
            nc.scalar.dma_start(out=pbsb[:], in_=pb.rearrange("(c p) b -> p c b", p=128))
            identf = singles.tile([128, 128], f32, name="identf")
            make_identity(nc, identf)
            identr = singles.tile([128, 128], f32r, name="identr")
            nc.vector.tensor_copy(identr[:], identf[:])

            for b in range(BPC):
                # ---- load natural granules (t on partitions) ----
                nat = []
                for q in range(NG):
                    nt = natp.tile([128, GT, D], f32r, tag="nat",
                                   name=f"nat_{b}_{q}")
                    nc.sync.dma_start(
                        out=nt[:],
                        in_=vals[b, q * GT * 128:(q + 1) * GT * 128, :]
                        .rearrange("(g p) d -> p g d", p=128))
                    nat.append(nt)

                # ---- transposes (q-outer) interleaved with main matmuls ----
                th = thp.tile([128, UC, T], bf16, tag="th", name=f"th_{b}")
                for h2 in range(2):
                    vth = vtp.tile([128, DC, 1024], f32r, tag="vt",
                                   name=f"vt_{b}_{h2}")
                    for q in (2 * h2, 2 * h2 + 1):
                        for dc in range(DC):
                            pst = ps_tr.tile([128, 512], f32r, tag="tr",
                                             name=f"pst_{b}_{dc}_{q}")
                            for j in range(GT):
                                nc.tensor.transpose(
                                    pst[:, j * 128:(j + 1) * 128],
                                    nat[q][:, j, dc * 128:(dc + 1) * 128],
                                    identr[:])
                            nc.vector.tensor_copy(
                                vth[:, dc, (q % 2) * 512:(q % 2) * 512 + 512],
                                pst[:])
                    for ut in range(UC):
                        psm = ps_main.tile([128, 1024], f32, tag="main",
                                           name=f"psm_{b}_{ut}_{h2}")
                        for dc in range(DC):
                            for c2 in range(2):
                                nc.tensor.matmul(
                                    psm[:, c2 * 512:(c2 + 1) * 512],
                                    w1sb[:, dc, ut * 128:(ut + 1) * 128],
                                    vth[:, dc, c2 * 512:(c2 + 1) * 512],
                                    start=(dc == 0), stop=(dc == DC - 1))
                        nc.scalar.activation(
                            out=th[:, ut, h2 * 1024:(h2 + 1) * 1024],
                            in_=psm[:], func=TANH,
                            bias=pbsb[:, ut, b:b + 1], scale=1.0)

                # ---- score on PE -> scr row [1, T] (raw scores) ----
                scr = small.tile([1, T], f32, tag="scr", name=f"scr_{b}")
                for tch in range(4):
                    pss = ps_sc.tile([1, 512], f32, tag="sc",
                                     name=f"pss_{b}_{tch}")
                    for uc in range(UC):
                        nc.tensor.matmul(
                            pss[:], vsb[:, uc:uc + 1],
                            th[:, uc, tch * 512:(tch + 1) * 512],
                            start=(uc == 0), stop=(uc == UC - 1))
                    nc.scalar.copy(scr[:, tch * 512:(tch + 1) * 512], pss[:])

                # ---- attn columns: 16 tiny [1,128] PE transposes + exp ----
                psa = ps_sc.tile([128, TT], f32, tag="sc", name=f"psa_{b}")
                for tt in range(TT):
                    nc.tensor.transpose(psa[:, tt:tt + 1],
                                        scr[0:1, tt * 128:(tt + 1) * 128],
                                        identf[0:1, 0:1])
                acol = small.tile([128, TT], f32r, tag="acol", name=f"acol_{b}")
                nc.scalar.activation(out=acol[:], in_=psa[:], func=EXP,
                                     bias=0.0, scale=1.0)

                # ---- row path: exp, sum, scale, attn out ----
                nc.scalar.activation(out=scr[:], in_=scr[:], func=EXP,
                                     bias=0.0, scale=1.0)
                sm = small.tile([1, 1], f32, tag="sm", name=f"sm_{b}")
                nc.vector.reduce_sum(sm[:], scr[:], axis=X)
                rsum = small.tile([1, 1], f32, tag="rsum", name=f"rsum_{b}")
                nc.vector.reciprocal(rsum[:], sm[:])
                nc.vector.tensor_scalar_mul(scr[:], scr[:], rsum[:])
                nc.scalar.dma_start(out=attn_o[b:b + 1, :], in_=scr[:])

                # ---- context (unnormalized, scaled by rsum in the copy) ----
                psc = ps_sc.tile([1, D], f32, tag="sc", name=f"psc_{b}")
                for tt in range(TT):
                    q, g = divmod(tt, GT)
                    nc.tensor.matmul(psc[:], acol[:, tt:tt + 1],
                                     nat[q][:, g, :],
                                     start=(tt == 0), stop=(tt == TT - 1))
                ctxr = small.tile([1, D], f32, tag="ctxr", name=f"ctxr_{b}")
                nc.scalar.activation(out=ctxr[:], in_=psc[:], func=COPY,
                                     bias=0.0, scale=rsum[:])
                nc.scalar.dma_start(out=ctx_o[b:b + 1, :], in_=ctxr[:])

    nc.compile()
    return nc


def _get_nc():
    if "nc" not in _cache:
        _cache["nc"] = _build()
    return _cache["nc"]


def kernel(query, values, W1, b1, W2, b2, V, bV):
    import ml_dtypes
    from concourse.bass_utils import run_bass_kernel_spmd

    query = np.asarray(query, dtype=np.float32)
    values = np.asarray(values, dtype=np.float32)
    W1 = np.asarray(W1, dtype=np.float32)
    b1 = np.asarray(b1, dtype=np.float32)
    W2 = np.asarray(W2, dtype=np.float32)
    b2 = np.asarray(b2, dtype=np.float32)
    V = np.asarray(V, dtype=np.float32)

    pq = query @ W2 + b2[None, :] + b1[None, :]   # [B, U]
    pb_full = np.ascontiguousarray(pq.T)          # [U, B]
    vbf = V[:, 0].astype(ml_dtypes.bfloat16)      # [U]

    in_maps = []
    for c in range(N_CORES):
        sl = slice(c * BPC, (c + 1) * BPC)
        in_maps.append({
            "vals": np.ascontiguousarray(values[sl]),
            "w1": W1,
            "vbf": vbf,
            "pb": np.ascontiguousarray(pb_full[:, sl]),
        })

    nc = _get_nc()
    trace = os.environ.get("BASS_KERNEL_TRACE") == "1"
    if trace:
        try:
            import tracehelper
            tracehelper.install()
        except Exception:
            trace = False
    res = run_bass_kernel_spmd(nc, in_maps, list(range(N_CORES)), trace=trace)
    _cache["last_exec_time_ns"] = res.exec_time_ns

    context = np.empty((B, D), dtype=np.float32)
    attn = np.empty((B, T, 1), dtype=np.float32)
    for c in range(N_CORES):
        sl = slice(c * BPC, (c + 1) * BPC)
        context[sl] = res.results[c]["ctx_o"]
        attn[sl] = res.results[c]["attn_o"][:, :, None]
    return (context, attn)
